# revision 3
# baseline (speedup 1.0000x reference)
"""ExaoneMoESparseMoEBlock Trainium2 kernel.

Strategy (expert-parallel over 8 NeuronCores):
  - Routing (gate matmul + biased grouped top-k) computed host-side in float64
    (selection margins >> fp32 noise, bit-stable vs the fp32 reference).
  - Tokens are dispatched host-side: for each expert, its tokens are gathered
    into a padded [C] slot array (C = 256 covers max expert load).
  - Each core holds 8 experts and runs the SiLU-gated MLP for its experts over
    their gathered tokens, entirely in a transposed layout ([feature, token]),
    weights stationary / tokens moving, so no on-device transposes are needed.
  - Matmuls run in float32r (TF32-like, 1 cyc/row at N>=256, ~1.4e-4 rel err).
  - The shared expert (IS=2048) is tensor-parallel sharded over the 8 cores
    (256 inter-dim slice each); each core emits a full [H, T] partial.
  - Host applies routing weights, scatter-adds expert outputs, and sums the
    shared partials.
"""

import os
import sys
import types

import numpy as np

T, H, E, K_TOP = 1024, 2048, 64, 8
G, TG = 8, 4
I_DIM, IS_DIM = 1024, 2048
SCALE = 2.5
N_CORES = 8
EPC = E // N_CORES       # experts per core
ISC = IS_DIM // N_CORES  # shared-expert intermediate slice per core
HC = H // 128            # 16 h-chunks
IC = I_DIM // 128        # 8 i-chunks

_LAST_RESULT = None      # BassKernelResults of the most recent run (for test.py)


def _install_ntff_shim():
    """Register the axon NTFF profile hook if the image's antenv lacks it.

    Lets BASS_TRACE=1 produce a perfetto trace + exec_time_ns. Harmless no-op
    when tracing is off or the axon .so is absent.
    """
    try:
        import antenv
        if "antenv.axon_hooks" in sys.modules:
            return
        mod = types.ModuleType("antenv.axon_hooks")
        mod._hook = None
        mod.set_axon_ntff_profile_hook = lambda h: setattr(mod, "_hook", h)
        mod.get_axon_ntff_profile_hook = lambda: mod._hook
        sys.modules["antenv.axon_hooks"] = mod
        antenv.axon_hooks = mod
        from trn_agent_boot.trn_boot import _ntff_profile_via_ctypes
        mod.set_axon_ntff_profile_hook(
            _ntff_profile_via_ctypes("/opt/axon/libaxon_pjrt.so")
        )
    except Exception:
        pass


def _routing(x, gate_w, e_bias):
    """float64 replica of the reference's sigmoid biased grouped top-k."""
    logits = x.astype(np.float64) @ gate_w.astype(np.float64)
    scores = 1.0 / (1.0 + np.exp(-logits))
    sb = scores + e_bias.astype(np.float64)[None, :]
    gsz = E // G
    gs = sb.reshape(T, G, gsz)
    top2 = np.sort(gs, axis=-1)[:, :, -2:].sum(-1)
    gidx = np.argsort(-top2, axis=-1, kind="stable")[:, :TG]
    gmask = np.zeros((T, G), bool)
    gmask[np.arange(T)[:, None], gidx] = True
    masked = np.where(np.repeat(gmask, gsz, axis=1), sb, -np.inf)
    idx = np.argsort(-masked, axis=-1, kind="stable")[:, :K_TOP]
    w = np.take_along_axis(scores, idx, axis=1).astype(np.float32)
    w = w / w.sum(-1, keepdims=True)
    return (w * np.float32(SCALE)).astype(np.float32), idx.astype(np.int64)


_KERNEL_CACHE = {}


def _build_kernel(C):
    """Build the per-core SPMD Bass program for token capacity C."""
    import concourse.bass as bass
    from concourse import bacc
    import concourse.mybir as mybir
    import concourse.tile as tile

    F32 = mybir.dt.float32
    F32R = mybir.dt.float32r
    ACT = mybir.ActivationFunctionType

    nc = bacc.Bacc("TRN2", target_bir_lowering=False, debug=False)

    xe_d = nc.dram_tensor("xe", [EPC, HC, 128, C], F32R, kind="ExternalInput")
    wg_d = nc.dram_tensor("wg", [EPC, 2, HC, 128, 512], F32R, kind="ExternalInput")
    wu_d = nc.dram_tensor("wu", [EPC, 2, HC, 128, 512], F32R, kind="ExternalInput")
    wd_d = nc.dram_tensor("wd", [EPC, 2, IC, 128, 1024], F32R, kind="ExternalInput")
    xt_d = nc.dram_tensor("xt", [HC, 128, T], F32R, kind="ExternalInput")
    wsg_d = nc.dram_tensor("wsg", [HC, 128, ISC], F32R, kind="ExternalInput")
    wsu_d = nc.dram_tensor("wsu", [HC, 128, ISC], F32R, kind="ExternalInput")
    wsd_d = nc.dram_tensor("wsd", [ISC // 128, 128, H], F32R, kind="ExternalInput")
    yr_d = nc.dram_tensor("yr", [EPC, HC, 128, C], F32, kind="ExternalOutput")
    ys_d = nc.dram_tensor("ys", [HC, 128, T], F32, kind="ExternalOutput")

    with tile.TileContext(nc) as tc:
        with (
            tc.tile_pool(name="wpool", bufs=3) as wpool,     # 32KB/part slots
            tc.tile_pool(name="xpool", bufs=2) as xpool,     # 16KB/part
            tc.tile_pool(name="sgpool", bufs=2) as sgpool,   # 8KB/part
            tc.tile_pool(name="apool", bufs=2) as apool,     # 8KB/part
            tc.tile_pool(name="opool", bufs=3) as opool,     # 4KB/part
            tc.tile_pool(name="xtpool", bufs=3) as xtpool,   # 4KB/part
            tc.tile_pool(name="pp", bufs=8, space="PSUM") as pp,
        ):
            # ---------------- shared expert (TP slice of IS) ----------------
            wsg_t = wpool.tile([128, HC, ISC], F32R, tag="w")
            nc.sync.dma_start(wsg_t[:], wsg_d.ap().rearrange("c p i -> p c i"))
            psg_s = [pp.tile([128, 512], F32, name="ps", tag="ps") for _ in range(4)]  # (is_t, nh)
            for hc in range(HC):
                xt_t = xtpool.tile([128, T], F32R)
                nc.sync.dma_start(xt_t[:], xt_d.ap()[hc])
                for it in range(2):
                    for nh in range(2):
                        nc.tensor.matmul(
                            psg_s[2 * it + nh][:],
                            wsg_t[:, hc, it * 128:(it + 1) * 128],
                            xt_t[:, nh * 512:(nh + 1) * 512],
                            start=(hc == 0), stop=(hc == HC - 1),
                        )
            sg_s = sgpool.tile([128, 2, T], F32, tag="sg")
            for it in range(2):
                for nh in range(2):
                    nc.scalar.activation(
                        sg_s[:, it, nh * 512:(nh + 1) * 512],
                        psg_s[2 * it + nh][:], ACT.Silu,
                    )

            wsu_t = wpool.tile([128, HC, ISC], F32R, tag="w")
            nc.sync.dma_start(wsu_t[:], wsu_d.ap().rearrange("c p i -> p c i"))
            psu_s = [pp.tile([128, 512], F32, name="ps", tag="ps") for _ in range(4)]
            for hc in range(HC):
                xt_t = xtpool.tile([128, T], F32R)
                nc.sync.dma_start(xt_t[:], xt_d.ap()[hc])
                for it in range(2):
                    for nh in range(2):
                        nc.tensor.matmul(
                            psu_s[2 * it + nh][:],
                            wsu_t[:, hc, it * 128:(it + 1) * 128],
                            xt_t[:, nh * 512:(nh + 1) * 512],
                            start=(hc == 0), stop=(hc == HC - 1),
                        )
            sa_s = apool.tile([128, 2, T], F32R, tag="a")
            for it in range(2):
                for nh in range(2):
                    nc.vector.tensor_mul(
                        sa_s[:, it, nh * 512:(nh + 1) * 512],
                        sg_s[:, it, nh * 512:(nh + 1) * 512],
                        psu_s[2 * it + nh][:],
                    )

            wsd_t = wpool.tile([128, ISC // 128, H], F32R, tag="w")
            nc.sync.dma_start(wsd_t[:], wsd_d.ap().rearrange("c p i -> p c i"))
            for ht in range(HC):
                psy = [pp.tile([128, 512], F32, name="ps", tag="ps") for _ in range(2)]
                for ic in range(2):
                    for nh in range(2):
                        nc.tensor.matmul(
                            psy[nh][:],
                            wsd_t[:, ic, ht * 128:(ht + 1) * 128],
                            sa_s[:, ic, nh * 512:(nh + 1) * 512],
                            start=(ic == 0), stop=(ic == 1),
                        )
                yo = opool.tile([128, T], F32, tag="o")
                nc.vector.tensor_copy(yo[:, 0:512], psy[0][:])
                nc.vector.tensor_copy(yo[:, 512:1024], psy[1][:])
                nc.sync.dma_start(ys_d.ap()[ht], yo[:])

            # ---------------- routed experts ----------------
            for e in range(EPC):
                xe_t = xpool.tile([128, HC, C], F32R, tag="xe")
                nc.sync.dma_start(
                    xe_t[:], xe_d.ap()[e].rearrange("c p n -> p c n")
                )
                sg_t = sgpool.tile([128, IC, C], F32, tag="sg")
                a_t = apool.tile([128, IC, C], F32R, tag="a")
                for ihalf in range(2):
                    wg_t = wpool.tile([128, HC, 512], F32R, tag="w")
                    nc.sync.dma_start(
                        wg_t[:], wg_d.ap()[e, ihalf].rearrange("c p i -> p c i")
                    )
                    for it in range(4):
                        psg = pp.tile([128, C], F32, name="ps", tag="ps")
                        for hc in range(HC):
                            nc.tensor.matmul(
                                psg[:],
                                wg_t[:, hc, it * 128:(it + 1) * 128],
                                xe_t[:, hc, :],
                                start=(hc == 0), stop=(hc == HC - 1),
                            )
                        nc.scalar.activation(
                            sg_t[:, ihalf * 4 + it, :], psg[:], ACT.Silu
                        )
                    wu_t = wpool.tile([128, HC, 512], F32R, tag="w")
                    nc.sync.dma_start(
                        wu_t[:], wu_d.ap()[e, ihalf].rearrange("c p i -> p c i")
                    )
                    for it in range(4):
                        psu = pp.tile([128, C], F32, name="ps", tag="ps")
                        for hc in range(HC):
                            nc.tensor.matmul(
                                psu[:],
                                wu_t[:, hc, it * 128:(it + 1) * 128],
                                xe_t[:, hc, :],
                                start=(hc == 0), stop=(hc == HC - 1),
                            )
                        nc.vector.tensor_mul(
                            a_t[:, ihalf * 4 + it, :],
                            sg_t[:, ihalf * 4 + it, :], psu[:],
                        )
                for hh in range(2):
                    wd_t = wpool.tile([128, IC, 1024], F32R, tag="w")
                    nc.sync.dma_start(
                        wd_t[:], wd_d.ap()[e, hh].rearrange("c p i -> p c i")
                    )
                    for ht in range(IC):
                        psy = pp.tile([128, C], F32, name="ps", tag="ps")
                        for ic in range(IC):
                            nc.tensor.matmul(
                                psy[:],
                                wd_t[:, ic, ht * 128:(ht + 1) * 128],
                                a_t[:, ic, :],
                                start=(ic == 0), stop=(ic == IC - 1),
                            )
                        yo = opool.tile([128, C], F32, tag="o")
                        nc.vector.tensor_copy(yo[:], psy[:])
                        nc.sync.dma_start(yr_d.ap()[e, hh * 8 + ht], yo[:])

    nc.compile()
    return nc


def kernel(hidden_states, gate_w, e_bias, w_gate, w_up, w_down,
           ws_gate, ws_up, ws_down):
    global _LAST_RESULT
    _install_ntff_shim()
    from concourse.bass_utils import run_bass_kernel_spmd

    x = np.ascontiguousarray(np.asarray(hidden_states, dtype=np.float32))
    gate_w = np.asarray(gate_w, dtype=np.float32)
    e_bias = np.asarray(e_bias, dtype=np.float32)
    w_gate = np.ascontiguousarray(np.asarray(w_gate, dtype=np.float32))
    w_up = np.ascontiguousarray(np.asarray(w_up, dtype=np.float32))
    w_down = np.ascontiguousarray(np.asarray(w_down, dtype=np.float32))
    ws_gate = np.ascontiguousarray(np.asarray(ws_gate, dtype=np.float32))
    ws_up = np.ascontiguousarray(np.asarray(ws_up, dtype=np.float32))
    ws_down = np.ascontiguousarray(np.asarray(ws_down, dtype=np.float32))

    w_route, idx = _routing(x, gate_w, e_bias)

    # per-expert token lists + per-slot routing weights
    tok = [np.nonzero((idx == e).any(axis=1))[0] for e in range(E)]
    wt = []
    for e in range(E):
        k_of_t = (idx[tok[e]] == e).argmax(axis=1)
        wt.append(w_route[tok[e], k_of_t])
    max_cnt = max(len(t) for t in tok)
    C = max(256, ((max_cnt + 127) // 128) * 128)

    if C not in _KERNEL_CACHE:
        _KERNEL_CACHE[C] = _build_kernel(C)
    nc = _KERNEL_CACHE[C]

    xt_l = np.ascontiguousarray(x.T).reshape(HC, 128, T)
    in_maps = []
    for c in range(N_CORES):
        es = slice(c * EPC, (c + 1) * EPC)
        xe_l = np.zeros((EPC, HC, 128, C), np.float32)
        for j, e in enumerate(range(c * EPC, (c + 1) * EPC)):
            buf = np.zeros((C, H), np.float32)
            buf[: len(tok[e])] = x[tok[e]]
            xe_l[j] = buf.T.reshape(HC, 128, C)
        wg_l = np.ascontiguousarray(
            w_gate[es].reshape(EPC, HC, 128, 2, 512).transpose(0, 3, 1, 2, 4))
        wu_l = np.ascontiguousarray(
            w_up[es].reshape(EPC, HC, 128, 2, 512).transpose(0, 3, 1, 2, 4))
        wd_l = np.ascontiguousarray(
            w_down[es].reshape(EPC, IC, 128, 2, 1024).transpose(0, 3, 1, 2, 4))
        wsg_l = np.ascontiguousarray(
            ws_gate[:, c * ISC:(c + 1) * ISC]).reshape(HC, 128, ISC)
        wsu_l = np.ascontiguousarray(
            ws_up[:, c * ISC:(c + 1) * ISC]).reshape(HC, 128, ISC)
        wsd_l = np.ascontiguousarray(
            ws_down[c * ISC:(c + 1) * ISC]).reshape(ISC // 128, 128, H)
        in_maps.append({
            "xe": xe_l, "wg": wg_l, "wu": wu_l, "wd": wd_l,
            "xt": xt_l, "wsg": wsg_l, "wsu": wsu_l, "wsd": wsd_l,
        })

    res = run_bass_kernel_spmd(nc, in_maps, core_ids=list(range(N_CORES)))
    _LAST_RESULT = res

    y = np.zeros((H, T), np.float32)
    for c in range(N_CORES):
        y += res.results[c]["ys"].reshape(H, T)
    out = np.ascontiguousarray(y.T)
    for c in range(N_CORES):
        yr = res.results[c]["yr"]
        for j, e in enumerate(range(c * EPC, (c + 1) * EPC)):
            cnt = len(tok[e])
            if cnt == 0:
                continue
            O = yr[j].reshape(H, C)[:, :cnt]
            out[tok[e]] += wt[e][:, None] * O.T
    return out
